# revision 1
# baseline (speedup 1.0000x reference)
"""FourierLinear Trainium2 kernel v3 — on-device tables + parity folding.

v2 computed  y = x @ (Tc^T diag(s) Bc - Ts^T diag(s) Bs) * 2^-16  with
full contractions k=4096 (stage 1) and f=2048 (stage 2).  v3 halves both
matmul contractions using cos(t + pi*a*h) = (-1)^(a*h) cos(t):

  stage 1 (k-fold):  cos(w_f (k'+2048h)) = (-1)^(a_f h) cos(w_f k'), and the
    sin term of the angle addition vanishes (sin(pi a h) = 0).  So
      u[f] = sum_{k'<2048} trig(w_f k') * (x[k'] +- x[k'+2048])
    with the sign picked by parity of a_f  ->  contraction 4096 -> 2048.

  stage 2 (l-fold):  columns l' and l'+2048 share partial sums:
      y[:, l']      = yE + yO        y[:, l'+2048] = yE - yO
    where yE / yO accumulate only even-b / odd-b frequencies
    ->  per-output contraction 2048 -> 1024 (each f feeds one of yE/yO).

  Both need frequencies grouped by (a%2, b%2): a free host-side reorder of
  the f summation order.  Each of the 4 groups (~512 +- 20 freqs) is padded
  with zero-spectrum dummies to GP=640 (5 chunks of 128) so the kernel
  structure is static:  group g owns f-chunks [5g, 5g+5), uses x-fold
  xp (g<2) or xm (g>=2) in stage 1, and accumulates into yE (g even-b:
  g in {0,2}) or yO (g in {1,3}) in stage 2.

Tables are generated on device exactly as in v2 (fp32 mult -> int32 cast,
&4095, ScalarE Sin on the int mask).  Upload: 8 MB fp16 xT per core +
~30 KB indices/spectrum.  Output fp16 (host upcasts).
"""

import math

import numpy as np

import concourse.mybir as mybir
import concourse.tile as tile
from concourse import bacc
from concourse.bass_utils import run_bass_kernel_spmd

N_CORES = 8
IN_F = 4096
OUT_F = 4096
NF = 2048
ROWS = 8192
M = ROWS // N_CORES   # 1024 rows per core
P = 128
KH = IN_F // 2        # 2048 folded k' range
KCH = KH // P         # 16 k'-chunks
LH = OUT_F // 2       # 2048 folded l' range
NT = 512
LTH = LH // NT        # 4 l'-tiles
MS = M // P           # 8 row blocks

SIN_SCALE = -(2.0 * math.pi) / 4096.0
SIN_BIAS = math.pi

LAST_RESULTS = None
_NC_CACHE = None


def _build_nc(gcs):
    # gcs: chunks per (a%2, b%2) parity group; group g owns f-chunks
    # [off[g], off[g+1]) and uses xp (g<2) / xm (g>=2) in stage 1,
    # yE (g in {0,2}) / yO (g in {1,3}) in stage 2.
    off = [0]
    for g in range(4):
        off.append(off[-1] + gcs[g])
    FC2 = off[4]
    NF2 = FC2 * P
    chunk_grp = []
    for g in range(4):
        chunk_grp += [g] * gcs[g]
    f32 = mybir.dt.float32
    f16 = mybir.dt.float16
    i32 = mybir.dt.int32
    Sin = mybir.ActivationFunctionType.Sin
    mult = mybir.AluOpType.mult
    band = mybir.AluOpType.bitwise_and
    add = mybir.AluOpType.add
    sub = mybir.AluOpType.subtract

    nc = bacc.Bacc(None)
    xT = nc.declare_dram_parameter("xT", [IN_F, M], f16, isOutput=False)
    idxi = nc.declare_dram_parameter("idxi", [1, NF2], i32, isOutput=False)
    idxoT = nc.declare_dram_parameter("idxoT", [P, FC2], f32, isOutput=False)
    scol = nc.declare_dram_parameter("scol", [P, FC2], f32, isOutput=False)
    out = nc.declare_dram_parameter("out", [M, OUT_F], f16, isOutput=True)

    xTp = xT[:].rearrange("(kc p) m -> p kc m", p=P)
    outp = out[:].rearrange("(ms p) n -> p ms n", p=P)

    with tile.TileContext(nc) as tc:
        with (
            tc.tile_pool(name="const", bufs=1) as cpool,
            tc.tile_pool(name="v", bufs=1) as vpool,
            tc.tile_pool(name="o", bufs=4) as opool,
        ):
            xp = cpool.tile([P, KCH, M], f16)       # x[k'] + x[k'+2048], 32 KB/part
            xm = cpool.tile([P, KCH, M], f16)       # x[k'] - x[k'+2048]
            idx_rep = cpool.tile([P, NF2], f32)     # a2 replicated to all partitions
            kmat = cpool.tile([P, KCH], f32)        # kmat[p,c] = 128c + p  (k' values)
            iota_l = cpool.tile([P, LH], f32)       # iota_l[p,i] = i       (l' values)
            idxo_sb = cpool.tile([P, FC2], f32)
            scol_sb = cpool.tile([P, FC2], f32)
            pibias = cpool.tile([P, 1], f32)
            vc = vpool.tile([P, FC2, M], f16)       # 40 KB/part each
            vs = vpool.tile([P, FC2, M], f16)

            nc.sync.dma_start(idxo_sb[:], idxoT[:])
            nc.sync.dma_start(scol_sb[:], scol[:])
            nc.gpsimd.iota(kmat[:], [[P, KCH]], base=0, channel_multiplier=1,
                           allow_small_or_imprecise_dtypes=True)
            nc.gpsimd.iota(iota_l[:], [[1, LH]], base=0, channel_multiplier=0,
                           allow_small_or_imprecise_dtypes=True)
            nc.vector.memset(pibias[:], SIN_BIAS)

            # ---- (first: tiny DMA, gates all table gen) broadcast a2 across partitions: ones^T @ idx
            with (
                tc.tile_pool(name="setup", bufs=1) as spool,
                tc.tile_pool(name="psb", bufs=1, space="PSUM") as psb,
            ):
                idxi_i = spool.tile([1, NF2], i32)
                idxi_f = spool.tile([1, NF2], f32)
                ones_sb = spool.tile([1, P], f32)
                nc.sync.dma_start(idxi_i[:], idxi[:])
                nc.vector.tensor_copy(out=idxi_f[:], in_=idxi_i[:])
                nc.vector.memset(ones_sb[:], 1.0)
                bps = psb.tile([P, NF2], f32)
                for t0 in range(0, NF2, NT):
                    t1 = min(t0 + NT, NF2)
                    nc.tensor.matmul(
                        bps[:, t0:t1],
                        ones_sb[:],
                        idxi_f[:, t0:t1],
                        start=True,
                        stop=True,
                    )
                nc.vector.tensor_copy(out=idx_rep[:], in_=bps[:])

            # ---- stream x in and fold: xp/xm = x_lo +- x_hi
            with tc.tile_pool(name="xs", bufs=4) as xsp:
                for kc in range(KCH):
                    lo = xsp.tile([P, M], f16, tag="lo")
                    hi = xsp.tile([P, M], f16, tag="hi")
                    nc.sync.dma_start(lo[:], xTp[:, kc, :])
                    nc.scalar.dma_start(hi[:], xTp[:, kc + KCH, :])
                    nc.vector.tensor_tensor(out=xp[:, kc, :], in0=lo[:], in1=hi[:],
                                            op=add)
                    nc.vector.tensor_tensor(out=xm[:, kc, :], in0=lo[:], in1=hi[:],
                                            op=sub)

            # ---- stage 1: u^T[f,m] = T_in^T @ xfold, scaled into vc/vs
            # group g (f-chunks 5g..5g+5) uses xp if g<2 else xm; scale by
            # s*2^-16 here so stage-2 psums combine without a final rescale.
            with (
                tc.tile_pool(name="t1f", bufs=3) as t1f,
                tc.tile_pool(name="t1h", bufs=3) as t1h,
                tc.tile_pool(name="ps1", bufs=4, space="PSUM") as ps1,
            ):
                for fb in range(FC2):
                    xf = xp if chunk_grp[fb] < 2 else xm
                    psc = ps1.tile([P, M], f32, tag="u", name=f"psc{fb}")
                    pss = ps1.tile([P, M], f32, tag="u", name=f"pss{fb}")
                    for kc in range(KCH):
                        prod_s = t1f.tile([P, P], i32, tag="ps")
                        prod_c = t1f.tile([P, P], i32, tag="pc")
                        tcs = t1h.tile([P, P], f16, tag="tc")
                        tss = t1h.tile([P, P], f16, tag="ts")
                        idx_sl = idx_rep[:, fb * P : (fb + 1) * P]
                        kv = kmat[:, kc : kc + 1]
                        nc.vector.tensor_scalar(prod_s[:], idx_sl, kv, None, mult)
                        nc.vector.tensor_scalar(prod_c[:], idx_sl, kv, 1024.0,
                                                mult, add)
                        nc.vector.tensor_scalar(prod_s[:], prod_s[:], 4095, None, band)
                        nc.vector.tensor_scalar(prod_c[:], prod_c[:], 4095, None, band)
                        nc.scalar.activation(tcs[:], prod_c[:], Sin,
                                             bias=pibias[:], scale=SIN_SCALE)
                        nc.scalar.activation(tss[:], prod_s[:], Sin,
                                             bias=pibias[:], scale=SIN_SCALE)
                        st, sp = kc == 0, kc == KCH - 1
                        nc.tensor.matmul(psc[:, 0:NT], tcs[:], xf[:, kc, 0:NT],
                                         start=st, stop=sp)
                        nc.tensor.matmul(psc[:, NT:M], tcs[:], xf[:, kc, NT:M],
                                         start=st, stop=sp)
                        nc.tensor.matmul(pss[:, 0:NT], tss[:], xf[:, kc, 0:NT],
                                         start=st, stop=sp)
                        nc.tensor.matmul(pss[:, NT:M], tss[:], xf[:, kc, NT:M],
                                         start=st, stop=sp)
                    nc.vector.tensor_scalar(vc[:, fb, :], psc[:],
                                            scol_sb[:, fb : fb + 1], 2.0 ** -16,
                                            mult, mult)
                    nc.vector.tensor_scalar(vs[:, fb, :], pss[:],
                                            scol_sb[:, fb : fb + 1], -(2.0 ** -16),
                                            mult, mult)

            # ---- stage 2: yE/yO over l' < 2048, y = yE +- yO
            # Phase the E/O accumulators in time instead of PSUM banks:
            # even-b chunks accumulate in all 8 banks, park yE in SBUF,
            # then odd-b chunks reuse the banks; combine yE +- psO.
            # Tables generate exactly once per (lt, fc).
            e_chunks = [fc for fc in range(FC2) if chunk_grp[fc] % 2 == 0]
            o_chunks = [fc for fc in range(FC2) if chunk_grp[fc] % 2 == 1]
            with (
                tc.tile_pool(name="t2f", bufs=3) as t2f,
                tc.tile_pool(name="t2h", bufs=3) as t2h,
                tc.tile_pool(name="ye", bufs=1) as yep,
                tc.tile_pool(name="ps2", bufs=8, space="PSUM") as ps2,
            ):
                def gen_tables(lt, fc):
                    prod2s = t2f.tile([P, NT], i32, tag="ps", name="prod2s")
                    prod2c = t2f.tile([P, NT], i32, tag="pc", name="prod2c")
                    bcs = t2h.tile([P, NT], f16, tag="bc", name="bcs")
                    bss = t2h.tile([P, NT], f16, tag="bs", name="bss")
                    il_sl = iota_l[:, lt * NT : (lt + 1) * NT]
                    ov = idxo_sb[:, fc : fc + 1]
                    nc.vector.tensor_scalar(prod2s[:], il_sl, ov, None, mult)
                    nc.gpsimd.tensor_scalar(prod2c[:], prod2s[:], 1024, None, add)
                    nc.vector.tensor_scalar(prod2s[:], prod2s[:], 4095, None, band)
                    nc.vector.tensor_scalar(prod2c[:], prod2c[:], 4095, None, band)
                    nc.scalar.activation(bcs[:], prod2c[:], Sin,
                                         bias=pibias[:], scale=SIN_SCALE)
                    nc.scalar.activation(bss[:], prod2s[:], Sin,
                                         bias=pibias[:], scale=SIN_SCALE)
                    return bcs, bss

                for lt in range(LTH):
                    yE = yep.tile([P, MS, NT], f32, tag="ye", name=f"yE{lt}")
                    psA = [ps2.tile([P, NT], f32, tag="y", name=f"psA{ms}")
                           for ms in range(MS)]
                    for fc in e_chunks:
                        bcs, bss = gen_tables(lt, fc)
                        for ms in range(MS):
                            nc.tensor.matmul(
                                psA[ms][:], vc[:, fc, ms * P : (ms + 1) * P],
                                bcs[:], start=(fc == e_chunks[0]), stop=False)
                            nc.tensor.matmul(
                                psA[ms][:], vs[:, fc, ms * P : (ms + 1) * P],
                                bss[:], start=False, stop=(fc == e_chunks[-1]))
                    for ms in range(MS):
                        nc.scalar.copy(out=yE[:, ms, :], in_=psA[ms][:])
                    psB = [ps2.tile([P, NT], f32, tag="y", name=f"psB{ms}")
                           for ms in range(MS)]
                    for fc in o_chunks:
                        bcs, bss = gen_tables(lt, fc)
                        for ms in range(MS):
                            nc.tensor.matmul(
                                psB[ms][:], vc[:, fc, ms * P : (ms + 1) * P],
                                bcs[:], start=(fc == o_chunks[0]), stop=False)
                            nc.tensor.matmul(
                                psB[ms][:], vs[:, fc, ms * P : (ms + 1) * P],
                                bss[:], start=False, stop=(fc == o_chunks[-1]))
                    for ms in range(MS):
                        olo = opool.tile([P, NT], f16, tag="olo", name="olo")
                        ohi = opool.tile([P, NT], f16, tag="ohi", name="ohi")
                        nc.vector.tensor_tensor(out=olo[:], in0=yE[:, ms, :],
                                                in1=psB[ms][:], op=add)
                        nc.vector.tensor_tensor(out=ohi[:], in0=yE[:, ms, :],
                                                in1=psB[ms][:], op=sub)
                        nc.scalar.dma_start(
                            outp[:, ms, lt * NT : (lt + 1) * NT], olo[:])
                        nc.scalar.dma_start(
                            outp[:, ms, LH + lt * NT : LH + (lt + 1) * NT],
                            ohi[:])
    nc.finalize()
    return nc


def _host_prep(x, spectrum, indices):
    x2 = np.asarray(x, dtype=np.float32).reshape(ROWS, IN_F)
    idx = np.asarray(indices, dtype=np.int64)
    s = np.asarray(spectrum, dtype=np.float32)
    a, b = idx[0], idx[1]

    # reference scatter is last-write-wins on duplicate (a,b) pairs
    keys = a * OUT_F + b
    _, first_of_reversed = np.unique(keys[::-1], return_index=True)
    keep = np.zeros(NF, dtype=bool)
    keep[NF - 1 - first_of_reversed] = True
    s_eff = np.where(keep, s, 0.0).astype(np.float32)

    # group frequencies by (a%2, b%2); pad each group to a whole number of
    # 128-chunks with zero-spectrum dummies whose indices keep the parity
    sels = [np.nonzero(((a % 2) == (g >> 1)) & ((b % 2) == (g & 1)))[0]
            for g in range(4)]
    gcs = [max(1, -(-len(sel) // P)) for sel in sels]   # chunks per group, >= 1
    FC2 = sum(gcs)
    NF2 = FC2 * P
    a2 = np.zeros(NF2, np.int64)
    b2 = np.zeros(NF2, np.int64)
    s2 = np.zeros(NF2, np.float32)
    o = 0
    for g in range(4):
        sel = sels[g]
        a2[o : o + len(sel)] = a[sel]
        b2[o : o + len(sel)] = b[sel]
        s2[o : o + len(sel)] = s_eff[sel]
        a2[o + len(sel) : o + gcs[g] * P] = g >> 1
        b2[o + len(sel) : o + gcs[g] * P] = g & 1
        o += gcs[g] * P

    idxi = np.ascontiguousarray(a2[None, :].astype(np.int32))                 # [1, NF2]
    idxoT = np.ascontiguousarray(b2.astype(np.float32).reshape(FC2, P).T)     # [P, FC2]
    scol = np.ascontiguousarray(s2.reshape(FC2, P).T)                         # [P, FC2]
    return x2, idxi, idxoT, scol, tuple(gcs)


def kernel(x, spectrum, indices):
    global _NC_CACHE, LAST_RESULTS
    x2, idxi, idxoT, scol, gcs = _host_prep(x, spectrum, indices)

    if _NC_CACHE is None or _NC_CACHE[0] != gcs:
        _NC_CACHE = (gcs, _build_nc(gcs))
    nc = _NC_CACHE[1]

    x16 = x2.astype(np.float16)   # cast once, then transpose half the bytes
    in_maps = [
        {
            "xT": np.ascontiguousarray(x16[j * M : (j + 1) * M].T),
            "idxi": idxi,
            "idxoT": idxoT,
            "scol": scol,
        }
        for j in range(N_CORES)
    ]
    res = run_bass_kernel_spmd(nc, in_maps, list(range(N_CORES)))
    LAST_RESULTS = res
    out = np.concatenate(
        [res.results[j]["out"].astype(np.float32) for j in range(N_CORES)], axis=0
    )
    return out.reshape(np.asarray(x).shape[:-1] + (OUT_F,))



# revision 3
# speedup vs baseline: 1.7883x; 1.7883x over previous
"""FourierLinear Trainium2 kernel v4 — host-precomputed trig tables.

v3 generated cos/sin tables on device (DVE int ops + ScalarE Sin +
GpSimd adds), which kept Vector/Scalar/GpSimd ~50% busy and stalled the
PE (60% busy, 1.0 ms measured).  The tables are x-independent constants,
so v4 precomputes them on the host and streams them from HBM, leaving
the device a pure two-stage fp16 matmul pipeline:

  stage 1 (k-parity fold):  u_f[m] = sum_{k'<2048} trig(w a_f k') xfold
    with xfold = x_lo +- x_hi picked by parity(a_f)
      -> psum f32, copied to SBUF as vc/vs = u * 2^-8 (fp16)
  stage 2 (l-parity fold):  yE/yO accumulate s_f 2^-8 (uc cos - us sin)
    over even-b / odd-b frequencies;  y[l'] = yE+yO, y[l'+2048] = yE-yO

The 2^-16 ifft2 norm (* 256 scale) is split 2^-8 at the stage-1 copy
and 2^-8 folded into the stage-2 tables so every fp16 tensor stays in
the normal range.  Frequencies are grouped by (a%2, b%2) and padded to
128-chunks exactly as in v3.

Per-core upload: 8 MB folded x + 37.8 MB tables (all fp16).  In-kernel
HBM reads ~77 GB/s per stage — far under the DMA roofline.  PE work:
2 stages x FC2*16*2 (or 4*FC2*8*2) matmuls of 512 free = ~1.18M cycles
~ 490 us at 2.4 GHz.
"""

import numpy as np

import concourse.mybir as mybir
import concourse.tile as tile
from concourse import bacc
from concourse.bass_utils import run_bass_kernel_spmd

N_CORES = 8
IN_F = 4096
OUT_F = 4096
NF = 2048
ROWS = 8192
M = ROWS // N_CORES   # 1024 rows per core
P = 128
KH = IN_F // 2        # 2048 folded k' range
KCH = KH // P         # 16 k'-chunks
LH = OUT_F // 2       # 2048 folded l' range
NT = 512
LTH = LH // NT        # 4 l'-tiles
MS = M // P           # 8 row blocks

LAST_RESULTS = None
_NC_CACHE = None


def _build_nc(gcs):
    # gcs: chunks per (a%2, b%2) parity group; group g owns f-chunks
    # [off[g], off[g+1]) and uses xp (g<2) / xm (g>=2) in stage 1,
    # yE (g in {0,2}) / yO (g in {1,3}) in stage 2.
    off = [0]
    for g in range(4):
        off.append(off[-1] + gcs[g])
    FC2 = off[4]
    chunk_grp = []
    for g in range(4):
        chunk_grp += [g] * gcs[g]
    f32 = mybir.dt.float32
    f16 = mybir.dt.float16
    mult = mybir.AluOpType.mult
    add = mybir.AluOpType.add
    sub = mybir.AluOpType.subtract

    nc = bacc.Bacc(None)
    xpT = nc.declare_dram_parameter("xpT", [KH, M], f16, isOutput=False)
    xmT = nc.declare_dram_parameter("xmT", [KH, M], f16, isOutput=False)
    t1c = nc.declare_dram_parameter("t1c", [FC2 * KCH * P, P], f16, isOutput=False)
    t1s = nc.declare_dram_parameter("t1s", [FC2 * KCH * P, P], f16, isOutput=False)
    t2c = nc.declare_dram_parameter("t2c", [FC2 * LTH * P, NT], f16, isOutput=False)
    t2s = nc.declare_dram_parameter("t2s", [FC2 * LTH * P, NT], f16, isOutput=False)
    out = nc.declare_dram_parameter("out", [M, OUT_F], f16, isOutput=True)

    xpp = xpT[:].rearrange("(kc p) m -> p kc m", p=P)
    xmp = xmT[:].rearrange("(kc p) m -> p kc m", p=P)
    t1cp = t1c[:].rearrange("(fb kc p) j -> p fb kc j", fb=FC2, kc=KCH, p=P)
    t1sp = t1s[:].rearrange("(fb kc p) j -> p fb kc j", fb=FC2, kc=KCH, p=P)
    t2cp = t2c[:].rearrange("(fc lt p) l -> p fc lt l", fc=FC2, lt=LTH, p=P)
    t2sp = t2s[:].rearrange("(fc lt p) l -> p fc lt l", fc=FC2, lt=LTH, p=P)
    outp = out[:].rearrange("(ms p) n -> p ms n", p=P)

    with tile.TileContext(nc) as tc:
        with (
            tc.tile_pool(name="v", bufs=1) as vpool,
            tc.tile_pool(name="o", bufs=4) as opool,
        ):
            vc = vpool.tile([P, FC2, M], f16)   # u_cos * 2^-8, 36 KB/part
            vs = vpool.tile([P, FC2, M], f16)

            # ---- stage 1: u^T[f,m] = T1^T @ xfold -> vc/vs
            with (
                tc.tile_pool(name="x", bufs=1) as xpool,
                tc.tile_pool(name="t1", bufs=3) as t1p,
                tc.tile_pool(name="ps1", bufs=4, space="PSUM") as ps1,
            ):
                xp = xpool.tile([P, KCH, M], f16)   # 32 KB/part
                xm = xpool.tile([P, KCH, M], f16)
                nc.sync.dma_start(xp[:], xpp[:])
                nc.scalar.dma_start(xm[:], xmp[:])
                for fb in range(FC2):
                    xf = xp if chunk_grp[fb] < 2 else xm
                    tcb = t1p.tile([P, KCH, P], f16, tag="tc")
                    tsb = t1p.tile([P, KCH, P], f16, tag="ts")
                    nc.sync.dma_start(tcb[:], t1cp[:, fb, :, :])
                    nc.scalar.dma_start(tsb[:], t1sp[:, fb, :, :])
                    psc = ps1.tile([P, M], f32, tag="u", name=f"psc{fb}")
                    pss = ps1.tile([P, M], f32, tag="u", name=f"pss{fb}")
                    for kc in range(KCH):
                        st, sp = kc == 0, kc == KCH - 1
                        nc.tensor.matmul(psc[:, 0:NT], tcb[:, kc, :],
                                         xf[:, kc, 0:NT], start=st, stop=sp)
                        nc.tensor.matmul(psc[:, NT:M], tcb[:, kc, :],
                                         xf[:, kc, NT:M], start=st, stop=sp)
                        nc.tensor.matmul(pss[:, 0:NT], tsb[:, kc, :],
                                         xf[:, kc, 0:NT], start=st, stop=sp)
                        nc.tensor.matmul(pss[:, NT:M], tsb[:, kc, :],
                                         xf[:, kc, NT:M], start=st, stop=sp)
                    nc.scalar.mul(vc[:, fb, :], psc[:], 2.0 ** -8)
                    nc.vector.tensor_scalar(vs[:, fb, :], pss[:], 2.0 ** -8,
                                            None, mult)

            # ---- stage 2: yE/yO over l' < 2048, y = yE +- yO
            # E accumulates in all 8 PSUM banks, parks in SBUF, then O
            # reuses the banks; combine yE +- psO.  The minus sign of the
            # sin term is folded into t2s on the host.
            e_chunks = [fc for fc in range(FC2) if chunk_grp[fc] % 2 == 0]
            o_chunks = [fc for fc in range(FC2) if chunk_grp[fc] % 2 == 1]
            with (
                tc.tile_pool(name="t2", bufs=6) as t2p,
                tc.tile_pool(name="ye", bufs=1) as yep,
                tc.tile_pool(name="ps2", bufs=8, space="PSUM") as ps2,
            ):
                def run_chunks(chunks, pstag):
                    pss = [ps2.tile([P, NT], f32, tag="y", name=f"{pstag}{ms}")
                           for ms in range(MS)]
                    for fc in chunks:
                        bc = t2p.tile([P, NT], f16, tag="bc")
                        bs = t2p.tile([P, NT], f16, tag="bs")
                        nc.sync.dma_start(bc[:], t2cp[:, fc, lt, :])
                        nc.gpsimd.dma_start(bs[:], t2sp[:, fc, lt, :])
                        for ms in range(MS):
                            nc.tensor.matmul(
                                pss[ms][:], vc[:, fc, ms * P : (ms + 1) * P],
                                bc[:], start=(fc == chunks[0]), stop=False)
                            nc.tensor.matmul(
                                pss[ms][:], vs[:, fc, ms * P : (ms + 1) * P],
                                bs[:], start=False, stop=(fc == chunks[-1]))
                    return pss

                for lt in range(LTH):
                    yE = yep.tile([P, MS, NT], f32, tag="ye", name=f"yE{lt}")
                    psA = run_chunks(e_chunks, f"psA{lt}_")
                    for ms in range(MS):
                        nc.scalar.copy(out=yE[:, ms, :], in_=psA[ms][:])
                    psB = run_chunks(o_chunks, f"psB{lt}_")
                    for ms in range(MS):
                        olo = opool.tile([P, NT], f16, tag="olo", name="olo")
                        ohi = opool.tile([P, NT], f16, tag="ohi", name="ohi")
                        nc.vector.tensor_tensor(out=olo[:], in0=yE[:, ms, :],
                                                in1=psB[ms][:], op=add)
                        nc.vector.tensor_tensor(out=ohi[:], in0=yE[:, ms, :],
                                                in1=psB[ms][:], op=sub)
                        nc.scalar.dma_start(
                            outp[:, ms, lt * NT : (lt + 1) * NT], olo[:])
                        nc.scalar.dma_start(
                            outp[:, ms, LH + lt * NT : LH + (lt + 1) * NT],
                            ohi[:])
    nc.finalize()
    return nc


def _host_prep(x, spectrum, indices):
    x2 = np.asarray(x, dtype=np.float32).reshape(ROWS, IN_F)
    idx = np.asarray(indices, dtype=np.int64)
    s = np.asarray(spectrum, dtype=np.float32)
    a, b = idx[0], idx[1]

    # reference scatter is last-write-wins on duplicate (a,b) pairs
    keys = a * OUT_F + b
    _, first_of_reversed = np.unique(keys[::-1], return_index=True)
    keep = np.zeros(NF, dtype=bool)
    keep[NF - 1 - first_of_reversed] = True
    s_eff = np.where(keep, s, 0.0).astype(np.float32)

    # group frequencies by (a%2, b%2); pad each group to a whole number of
    # 128-chunks with zero-spectrum dummies whose indices keep the parity
    sels = [np.nonzero(((a % 2) == (g >> 1)) & ((b % 2) == (g & 1)))[0]
            for g in range(4)]
    gcs = [max(1, -(-len(sel) // P)) for sel in sels]   # chunks per group, >= 1
    FC2 = sum(gcs)
    NF2 = FC2 * P
    a2 = np.zeros(NF2, np.int64)
    b2 = np.zeros(NF2, np.int64)
    s2 = np.zeros(NF2, np.float32)
    o = 0
    for g in range(4):
        sel = sels[g]
        a2[o : o + len(sel)] = a[sel]
        b2[o : o + len(sel)] = b[sel]
        s2[o : o + len(sel)] = s_eff[sel]
        a2[o + len(sel) : o + gcs[g] * P] = g >> 1
        b2[o + len(sel) : o + gcs[g] * P] = g & 1
        o += gcs[g] * P

    w = 2.0 * np.pi / 4096.0
    kk = np.arange(KH)
    ll = np.arange(LH)
    # stage 1 tables [k', f] -> [fb, kc, p(k'), j(f)]
    ph1 = (a2[None, :] * kk[:, None]) % 4096
    t1c_full = np.cos(w * ph1, dtype=np.float32)
    t1s_full = np.sin(w * ph1, dtype=np.float32)
    t1c = np.ascontiguousarray(
        t1c_full.reshape(KCH, P, FC2, P).transpose(2, 0, 1, 3)
        .reshape(FC2 * KCH * P, P).astype(np.float16))
    t1s = np.ascontiguousarray(
        t1s_full.reshape(KCH, P, FC2, P).transpose(2, 0, 1, 3)
        .reshape(FC2 * KCH * P, P).astype(np.float16))
    # stage 2 tables [f, l'] with s*2^-8 folded in (sin term negated so the
    # psum accumulation is a pure add) -> [fc, lt, p(f), l']
    ph2 = (b2[:, None] * ll[None, :]) % 4096
    sc = (s2 * 2.0 ** -8)[:, None]
    t2c_full = np.cos(w * ph2, dtype=np.float32) * sc
    t2s_full = np.sin(w * ph2, dtype=np.float32) * (-sc)
    t2c = np.ascontiguousarray(
        t2c_full.reshape(FC2, P, LTH, NT).transpose(0, 2, 1, 3)
        .reshape(FC2 * LTH * P, NT).astype(np.float16))
    t2s = np.ascontiguousarray(
        t2s_full.reshape(FC2, P, LTH, NT).transpose(0, 2, 1, 3)
        .reshape(FC2 * LTH * P, NT).astype(np.float16))

    xp16 = (x2[:, :KH] + x2[:, KH:]).astype(np.float16)
    xm16 = (x2[:, :KH] - x2[:, KH:]).astype(np.float16)
    return xp16, xm16, t1c, t1s, t2c, t2s, tuple(gcs)


def kernel(x, spectrum, indices):
    global _NC_CACHE, LAST_RESULTS
    xp16, xm16, t1c, t1s, t2c, t2s, gcs = _host_prep(x, spectrum, indices)

    if _NC_CACHE is None or _NC_CACHE[0] != gcs:
        _NC_CACHE = (gcs, _build_nc(gcs))
    nc = _NC_CACHE[1]

    in_maps = [
        {
            "xpT": np.ascontiguousarray(xp16[j * M : (j + 1) * M].T),
            "xmT": np.ascontiguousarray(xm16[j * M : (j + 1) * M].T),
            "t1c": t1c,
            "t1s": t1s,
            "t2c": t2c,
            "t2s": t2s,
        }
        for j in range(N_CORES)
    ]
    res = run_bass_kernel_spmd(nc, in_maps, list(range(N_CORES)))
    LAST_RESULTS = res
    out = np.concatenate(
        [res.results[j]["out"].astype(np.float32) for j in range(N_CORES)], axis=0
    )
    return out.reshape(np.asarray(x).shape[:-1] + (OUT_F,))


# revision 8
# speedup vs baseline: 1.8370x; 1.0272x over previous
"""FourierLinear Trainium2 kernel v4 — host-precomputed trig tables.

v3 generated cos/sin tables on device (DVE int ops + ScalarE Sin +
GpSimd adds), which kept Vector/Scalar/GpSimd ~50% busy and stalled the
PE (60% busy, 1.0 ms measured).  The tables are x-independent constants,
so v4 precomputes them on the host and streams them from HBM, leaving
the device a pure two-stage fp16 matmul pipeline:

  stage 1 (k-parity fold):  u_f[m] = sum_{k'<2048} trig(w a_f k') xfold
    with xfold = x_lo +- x_hi picked by parity(a_f)
      -> psum f32, copied to SBUF as vc/vs = u * 2^-8 (fp16)
  stage 2 (l-parity fold):  yE/yO accumulate s_f 2^-8 (uc cos - us sin)
    over even-b / odd-b frequencies;  y[l'] = yE+yO, y[l'+2048] = yE-yO

The 2^-16 ifft2 norm (* 256 scale) is split 2^-8 at the stage-1 copy
and 2^-8 folded into the stage-2 tables so every fp16 tensor stays in
the normal range.  Frequencies are grouped by (a%2, b%2) and padded to
128-chunks exactly as in v3.

Per-core upload: 8 MB folded x + 37.8 MB tables (all fp16).  In-kernel
HBM reads ~77 GB/s per stage — far under the DMA roofline.  PE work:
2 stages x FC2*16*2 (or 4*FC2*8*2) matmuls of 512 free = ~1.18M cycles
~ 490 us at 2.4 GHz.
"""

import numpy as np

import concourse.mybir as mybir
import concourse.tile as tile
from concourse import bacc
from concourse.bass_utils import run_bass_kernel_spmd

N_CORES = 8
IN_F = 4096
OUT_F = 4096
NF = 2048
ROWS = 8192
M = ROWS // N_CORES   # 1024 rows per core
P = 128
KH = IN_F // 2        # 2048 folded k' range
KCH = KH // P         # 16 k'-chunks
LH = OUT_F // 2       # 2048 folded l' range
NT = 512
LTH = LH // NT        # 4 l'-tiles
MS = M // P           # 8 row blocks

LAST_RESULTS = None
_NC_CACHE = None


def _build_nc(gcs):
    # gcs: chunks per (a%2, b%2) parity group; group g owns f-chunks
    # [off[g], off[g+1]) and uses xp (g<2) / xm (g>=2) in stage 1,
    # yE (g in {0,2}) / yO (g in {1,3}) in stage 2.
    off = [0]
    for g in range(4):
        off.append(off[-1] + gcs[g])
    FC2 = off[4]
    chunk_grp = []
    for g in range(4):
        chunk_grp += [g] * gcs[g]
    f32 = mybir.dt.float32
    f16 = mybir.dt.float16
    mult = mybir.AluOpType.mult
    add = mybir.AluOpType.add
    sub = mybir.AluOpType.subtract

    nc = bacc.Bacc(None)
    xpT = nc.declare_dram_parameter("xpT", [KH, M], f16, isOutput=False)
    xmT = nc.declare_dram_parameter("xmT", [KH, M], f16, isOutput=False)
    t1c = nc.declare_dram_parameter("t1c", [FC2 * KCH * P, P], f16, isOutput=False)
    t1s = nc.declare_dram_parameter("t1s", [FC2 * KCH * P, P], f16, isOutput=False)
    t2c = nc.declare_dram_parameter("t2c", [FC2 * LTH * P, NT], f16, isOutput=False)
    t2s = nc.declare_dram_parameter("t2s", [FC2 * LTH * P, NT], f16, isOutput=False)
    out = nc.declare_dram_parameter("out", [M, OUT_F], f16, isOutput=True)

    xpp = xpT[:].rearrange("(kc p) m -> p kc m", p=P)
    xmp = xmT[:].rearrange("(kc p) m -> p kc m", p=P)
    t1cp = t1c[:].rearrange("(fb kc p) j -> p fb kc j", fb=FC2, kc=KCH, p=P)
    t1sp = t1s[:].rearrange("(fb kc p) j -> p fb kc j", fb=FC2, kc=KCH, p=P)
    t2cp = t2c[:].rearrange("(fc lt p) l -> p fc lt l", fc=FC2, lt=LTH, p=P)
    t2sp = t2s[:].rearrange("(fc lt p) l -> p fc lt l", fc=FC2, lt=LTH, p=P)
    outp = out[:].rearrange("(ms p) n -> p ms n", p=P)

    with tile.TileContext(nc) as tc:
        with (
            tc.tile_pool(name="v", bufs=1) as vpool,
            tc.tile_pool(name="o", bufs=4) as opool,
        ):
            vc = vpool.tile([P, FC2, M], f16)   # u_cos * 2^-8, 36 KB/part
            vs = vpool.tile([P, FC2, M], f16)

            # ---- stage 1: u^T[f,m] = T1^T @ xfold -> vc/vs
            with (
                tc.tile_pool(name="x", bufs=1) as xpool,
                tc.tile_pool(name="t1", bufs=3) as t1p,
                tc.tile_pool(name="ps1", bufs=4, space="PSUM") as ps1,
            ):
                xp = xpool.tile([P, KCH, M], f16)   # 32 KB/part
                xm = xpool.tile([P, KCH, M], f16)
                # First fb's tables go out first so the PE can start ~4us in;
                # x streams per-kc chunk behind them (xp feeds fb 0.., xm is
                # not needed until the a-odd groups halfway through stage 1).
                tcb0 = t1p.tile([P, KCH, P], f16, tag="tc")
                tsb0 = t1p.tile([P, KCH, P], f16, tag="ts")
                nc.sync.dma_start(tcb0[:], t1cp[:, 0, :, :])
                nc.scalar.dma_start(tsb0[:], t1sp[:, 0, :, :])
                for kc in range(KCH):
                    eng = nc.sync if kc % 2 == 0 else nc.scalar
                    eng.dma_start(xp[:, kc, :], xpp[:, kc, :])
                for kc in range(KCH):
                    eng = nc.sync if kc % 2 == 0 else nc.scalar
                    eng.dma_start(xm[:, kc, :], xmp[:, kc, :])
                for fb in range(FC2):
                    xf = xp if chunk_grp[fb] < 2 else xm
                    if fb == 0:
                        tcb, tsb = tcb0, tsb0
                    else:
                        tcb = t1p.tile([P, KCH, P], f16, tag="tc")
                        tsb = t1p.tile([P, KCH, P], f16, tag="ts")
                        nc.sync.dma_start(tcb[:], t1cp[:, fb, :, :])
                        nc.scalar.dma_start(tsb[:], t1sp[:, fb, :, :])
                    psc = ps1.tile([P, M], f32, tag="u", name=f"psc{fb}")
                    pss = ps1.tile([P, M], f32, tag="u", name=f"pss{fb}")
                    for kc in range(KCH):
                        st, sp = kc == 0, kc == KCH - 1
                        nc.tensor.matmul(psc[:, 0:NT], tcb[:, kc, :],
                                         xf[:, kc, 0:NT], start=st, stop=sp)
                        nc.tensor.matmul(psc[:, NT:M], tcb[:, kc, :],
                                         xf[:, kc, NT:M], start=st, stop=sp)
                        nc.tensor.matmul(pss[:, 0:NT], tsb[:, kc, :],
                                         xf[:, kc, 0:NT], start=st, stop=sp)
                        nc.tensor.matmul(pss[:, NT:M], tsb[:, kc, :],
                                         xf[:, kc, NT:M], start=st, stop=sp)
                    nc.scalar.mul(vc[:, fb, :], psc[:], 2.0 ** -8)
                    nc.vector.tensor_scalar(vs[:, fb, :], pss[:], 2.0 ** -8,
                                            None, mult)

            # ---- stage 2: yE/yO over l' < 2048, y = yE +- yO
            # E accumulates in all 8 PSUM banks, parks in SBUF, then O
            # reuses the banks; combine yE +- psO.  The minus sign of the
            # sin term is folded into t2s on the host.
            e_chunks = [fc for fc in range(FC2) if chunk_grp[fc] % 2 == 0]
            o_chunks = [fc for fc in range(FC2) if chunk_grp[fc] % 2 == 1]
            with (
                tc.tile_pool(name="t2", bufs=8) as t2p,
                tc.tile_pool(name="ye", bufs=1) as yep,
                tc.tile_pool(name="ps2", bufs=8, space="PSUM") as ps2,
            ):
                def run_chunks(chunks, pstag):
                    pss = [ps2.tile([P, NT], f32, tag="y", name=f"{pstag}{ms}")
                           for ms in range(MS)]
                    for fc in chunks:
                        bc = t2p.tile([P, NT], f16, tag="bc")
                        bs = t2p.tile([P, NT], f16, tag="bs")
                        nc.sync.dma_start(bc[:], t2cp[:, fc, lt, :])
                        nc.gpsimd.dma_start(bs[:], t2sp[:, fc, lt, :])
                        for ms in range(MS):
                            nc.tensor.matmul(
                                pss[ms][:], vc[:, fc, ms * P : (ms + 1) * P],
                                bc[:], start=(fc == chunks[0]), stop=False)
                            nc.tensor.matmul(
                                pss[ms][:], vs[:, fc, ms * P : (ms + 1) * P],
                                bs[:], start=False, stop=(fc == chunks[-1]))
                    return pss

                for lt in range(LTH):
                    yE = yep.tile([P, MS, NT], f32, tag="ye", name=f"yE{lt}")
                    psA = run_chunks(e_chunks, f"psA{lt}_")
                    for ms in range(MS):
                        if ms % 2 == 0:
                            nc.scalar.copy(out=yE[:, ms, :], in_=psA[ms][:])
                        else:
                            nc.vector.tensor_copy(out=yE[:, ms, :],
                                                  in_=psA[ms][:])
                    psB = run_chunks(o_chunks, f"psB{lt}_")
                    for ms in range(MS):
                        olo = opool.tile([P, NT], f16, tag="olo", name="olo")
                        ohi = opool.tile([P, NT], f16, tag="ohi", name="ohi")
                        nc.vector.tensor_tensor(out=olo[:], in0=yE[:, ms, :],
                                                in1=psB[ms][:], op=add)
                        nc.vector.tensor_tensor(out=ohi[:], in0=yE[:, ms, :],
                                                in1=psB[ms][:], op=sub)
                        nc.scalar.dma_start(
                            outp[:, ms, lt * NT : (lt + 1) * NT], olo[:])
                        nc.sync.dma_start(
                            outp[:, ms, LH + lt * NT : LH + (lt + 1) * NT],
                            ohi[:])
    nc.finalize()
    return nc


def _host_prep(x, spectrum, indices):
    x2 = np.asarray(x, dtype=np.float32).reshape(ROWS, IN_F)
    idx = np.asarray(indices, dtype=np.int64)
    s = np.asarray(spectrum, dtype=np.float32)
    a, b = idx[0], idx[1]

    # reference scatter is last-write-wins on duplicate (a,b) pairs
    keys = a * OUT_F + b
    _, first_of_reversed = np.unique(keys[::-1], return_index=True)
    keep = np.zeros(NF, dtype=bool)
    keep[NF - 1 - first_of_reversed] = True
    s_eff = np.where(keep, s, 0.0).astype(np.float32)

    # group frequencies by (a%2, b%2); pad each group to a whole number of
    # 128-chunks with zero-spectrum dummies whose indices keep the parity
    sels = [np.nonzero(((a % 2) == (g >> 1)) & ((b % 2) == (g & 1)))[0]
            for g in range(4)]
    gcs = [max(1, -(-len(sel) // P)) for sel in sels]   # chunks per group, >= 1
    FC2 = sum(gcs)
    NF2 = FC2 * P
    a2 = np.zeros(NF2, np.int64)
    b2 = np.zeros(NF2, np.int64)
    s2 = np.zeros(NF2, np.float32)
    o = 0
    for g in range(4):
        sel = sels[g]
        a2[o : o + len(sel)] = a[sel]
        b2[o : o + len(sel)] = b[sel]
        s2[o : o + len(sel)] = s_eff[sel]
        a2[o + len(sel) : o + gcs[g] * P] = g >> 1
        b2[o + len(sel) : o + gcs[g] * P] = g & 1
        o += gcs[g] * P

    w = 2.0 * np.pi / 4096.0
    kk = np.arange(KH)
    ll = np.arange(LH)
    # stage 1 tables [k', f] -> [fb, kc, p(k'), j(f)]
    ph1 = (a2[None, :] * kk[:, None]) % 4096
    t1c_full = np.cos(w * ph1, dtype=np.float32)
    t1s_full = np.sin(w * ph1, dtype=np.float32)
    t1c = np.ascontiguousarray(
        t1c_full.reshape(KCH, P, FC2, P).transpose(2, 0, 1, 3)
        .reshape(FC2 * KCH * P, P).astype(np.float16))
    t1s = np.ascontiguousarray(
        t1s_full.reshape(KCH, P, FC2, P).transpose(2, 0, 1, 3)
        .reshape(FC2 * KCH * P, P).astype(np.float16))
    # stage 2 tables [f, l'] with s*2^-8 folded in (sin term negated so the
    # psum accumulation is a pure add) -> [fc, lt, p(f), l']
    ph2 = (b2[:, None] * ll[None, :]) % 4096
    sc = (s2 * 2.0 ** -8)[:, None]
    t2c_full = np.cos(w * ph2, dtype=np.float32) * sc
    t2s_full = np.sin(w * ph2, dtype=np.float32) * (-sc)
    t2c = np.ascontiguousarray(
        t2c_full.reshape(FC2, P, LTH, NT).transpose(0, 2, 1, 3)
        .reshape(FC2 * LTH * P, NT).astype(np.float16))
    t2s = np.ascontiguousarray(
        t2s_full.reshape(FC2, P, LTH, NT).transpose(0, 2, 1, 3)
        .reshape(FC2 * LTH * P, NT).astype(np.float16))

    xp16 = (x2[:, :KH] + x2[:, KH:]).astype(np.float16)
    xm16 = (x2[:, :KH] - x2[:, KH:]).astype(np.float16)
    return xp16, xm16, t1c, t1s, t2c, t2s, tuple(gcs)


def kernel(x, spectrum, indices):
    global _NC_CACHE, LAST_RESULTS
    xp16, xm16, t1c, t1s, t2c, t2s, gcs = _host_prep(x, spectrum, indices)

    if _NC_CACHE is None or _NC_CACHE[0] != gcs:
        _NC_CACHE = (gcs, _build_nc(gcs))
    nc = _NC_CACHE[1]

    in_maps = [
        {
            "xpT": np.ascontiguousarray(xp16[j * M : (j + 1) * M].T),
            "xmT": np.ascontiguousarray(xm16[j * M : (j + 1) * M].T),
            "t1c": t1c,
            "t1s": t1s,
            "t2c": t2c,
            "t2s": t2s,
        }
        for j in range(N_CORES)
    ]
    res = run_bass_kernel_spmd(nc, in_maps, list(range(N_CORES)))
    LAST_RESULTS = res
    out = np.concatenate(
        [res.results[j]["out"].astype(np.float32) for j in range(N_CORES)], axis=0
    )
    return out.reshape(np.asarray(x).shape[:-1] + (OUT_F,))
